# revision 3
# baseline (speedup 1.0000x reference)
"""BNN MLP (784 -> 2048 -> 2048 -> 2048 -> 10, sign activations) on 8 TRN2 cores.

Strategy:
  - Data-parallel: batch 16384 sharded 2048/core; weights replicated.
  - fc1 (real-valued x @ sign(W1).T): x split into fp16 hi+lo (captures fp32
    to ~2^-23); both passes concatenated along K (784+784 -> 1664 padded) and
    run as 13 fp16 matmul chunks accumulating in fp32 PSUM.
  - BatchNorm + hardtanh + sign folds into a per-feature threshold computed
    on host in fp64: sign(bn(h)) == sign(h - t). Layer outputs are written as
    +-1 (ACT engine Sign) or {0,1} (DVE is_ge) in fp8e4 -- exactly
    representable -- alternating engines to balance the epilogue load.
  - fc2/fc3: inputs and weights are +-1/{0,1} in fp8e4 => products and fp32
    PSUM accumulation are exact integers. Uses DoubleRow perf mode (2 K-tiles
    per pass, 0.5 cycles/row). The {0,1} representation is corrected
    algebraically via row-sum constants folded into the next threshold.
  - fc4 + log_softmax on device: feature-major logits, exact bias add, PE
    transpose to batch-major, ACT Exp/Ln for log_softmax.
"""

import sys

sys.path.insert(0, "/opt/trn_rl_repo")

from contextlib import ExitStack

import ml_dtypes
import numpy as np

import concourse.bass as bass
import concourse.mybir as mybir
import concourse.tile as tile
from concourse import bacc
from concourse.bass_utils import run_bass_kernel_spmd
from concourse.masks import make_identity

BN_EPS = 1e-5
N_CORES = 8
B, D_IN, H, C = 16384, 784, 2048, 10
BS = B // N_CORES            # 2048 batch rows per core
NB = 512                     # moving free dim per matmul
NBG = BS // NB               # 4 batch groups per core
KC1 = 13                     # fc1 k-chunks: 2*784=1568 padded to 13*128=1664
K1P = KC1 * 128
FH = H // 128                # 16 feature tiles
DP2 = H // 256               # 8 DoubleRow k-pair chunks for fc2/fc3
F8 = mybir.dt.float8e4
F16 = mybir.dt.float16
F32 = mybir.dt.float32

_CACHE = {}


def _build_program(do_compile=True):
    nc = bacc.Bacc("TRN2", target_bir_lowering=False, debug=False,
                   num_devices=N_CORES)

    xcat = nc.dram_tensor("xcat", [KC1, 128, BS], F16, kind="ExternalInput").ap()
    w1 = nc.dram_tensor("w1", [KC1, 128, H], F16, kind="ExternalInput").ap()
    w2 = nc.dram_tensor("w2", [DP2, 128, 2, H], F8, kind="ExternalInput").ap()
    w3 = nc.dram_tensor("w3", [DP2, 128, 2, H], F8, kind="ExternalInput").ap()
    w4 = nc.dram_tensor("w4", [FH, 128, C], F8, kind="ExternalInput").ap()
    # negated thresholds (ACT Sign bias) for layers 1,3; threshold for layer 2
    nu1 = nc.dram_tensor("nu1", [128, FH], F32, kind="ExternalInput").ap()
    u2 = nc.dram_tensor("u2", [128, FH], F32, kind="ExternalInput").ap()
    nu3 = nc.dram_tensor("nu3", [128, FH], F32, kind="ExternalInput").ap()
    b4 = nc.dram_tensor("b4", [C, 1], F32, kind="ExternalInput").ap()
    out = nc.dram_tensor("out", [BS, C], F32, kind="ExternalOutput").ap()

    with tile.TileContext(nc) as tc, ExitStack() as ctx:
        const = ctx.enter_context(tc.tile_pool(name="const", bufs=1))
        pwa = ctx.enter_context(tc.tile_pool(name="pwa", bufs=1))
        pwb = ctx.enter_context(tc.tile_pool(name="pwb", bufs=1))
        px = ctx.enter_context(tc.tile_pool(name="px", bufs=2))
        psa = ctx.enter_context(tc.tile_pool(name="psa", bufs=1))
        psb = ctx.enter_context(tc.tile_pool(name="psb", bufs=1))
        small = ctx.enter_context(tc.tile_pool(name="small", bufs=2))
        pmm = ctx.enter_context(tc.tile_pool(name="pmm", bufs=4, space="PSUM"))
        p4p = ctx.enter_context(tc.tile_pool(name="p4p", bufs=2, space="PSUM"))
        ptp = ctx.enter_context(tc.tile_pool(name="ptp", bufs=2, space="PSUM"))

        ident = const.tile([128, 128], F32)
        make_identity(nc, ident[:])
        nu1_sb = const.tile([128, FH], F32)
        u2_sb = const.tile([128, FH], F32)
        nu3_sb = const.tile([128, FH], F32)
        b4_sb = const.tile([C, 1], F32)
        w4_sb = const.tile([128, FH, C], F8)
        nc.sync.dma_start(nu1_sb[:], nu1[:])
        nc.sync.dma_start(u2_sb[:], u2[:])
        nc.sync.dma_start(nu3_sb[:], nu3[:])
        nc.sync.dma_start(b4_sb[:], b4[:])
        nc.sync.dma_start(w4_sb[:], w4.rearrange("k p m -> p k m"))

        w1_sb = pwa.tile([128, KC1, H], F16, tag="wa")
        nc.sync.dma_start(w1_sb[:], w1.rearrange("k p m -> p k m"))
        w2_sb = pwb.tile([128, DP2, 2, H], F8, tag="wb")
        nc.sync.dma_start(w2_sb[:], w2.rearrange("d p j m -> p d j m"))

        s1 = psa.tile([128, FH, BS], F8, tag="sa")   # +-1, layer-1 output
        s2 = psb.tile([128, FH, BS], F8, tag="sb")   # {0,1}, layer-2 output

        # ---- fc1: raw1 = xcat.T @ w1cat; s1 = Sign(raw1 - u1) (ACT) ----
        for bg in range(NBG):
            bsl = bass.ts(bg, NB)
            xt = px.tile([128, KC1, NB], F16, tag="x")
            nc.sync.dma_start(xt[:], xcat[:, :, bsl].rearrange("k p b -> p k b"))
            for f in range(FH):
                fsl = bass.ts(f, 128)
                p = pmm.tile([128, NB], F32, tag="mm")
                for kc in range(KC1):
                    nc.tensor.matmul(p[:], w1_sb[:, kc, fsl], xt[:, kc, :],
                                     start=(kc == 0), stop=(kc == KC1 - 1))
                nc.scalar.activation(s1[:, f, bsl], p[:],
                                     mybir.ActivationFunctionType.Sign,
                                     bias=nu1_sb[:, f:f + 1], scale=1.0)

        # w1 slot is reused for w3 (DMA starts once fc1 finishes with w1)
        w3_sb = pwa.tile([128, DP2, 2, H], F8, tag="wa")
        nc.sync.dma_start(w3_sb[:], w3.rearrange("d p j m -> p d j m"))

        # ---- fc2: raw2 = s1(+-1) @ sign(W2).T; s2 = (raw2 >= u2) (DVE) ----
        for bg in range(NBG):
            bsl = bass.ts(bg, NB)
            for f in range(FH):
                fsl = bass.ts(f, 128)
                p = pmm.tile([128, NB], F32, tag="mm")
                for d in range(DP2):
                    nc.tensor.matmul(p[:], w2_sb[:, d, :, fsl],
                                     s1[:, 2 * d:2 * d + 2, bsl],
                                     start=(d == 0), stop=(d == DP2 - 1),
                                     perf_mode=mybir.MatmulPerfMode.DoubleRow)
                nc.vector.tensor_scalar(s2[:, f, bsl], p[:], u2_sb[:, f:f + 1],
                                        None, mybir.AluOpType.is_ge)

        # s1 slot reused for layer-3 output (+-1)
        s3 = psa.tile([128, FH, BS], F8, tag="sa")

        # ---- fc3: raw3 = s2({0,1}) @ sign(W3).T; s3 = Sign(raw3 - u3) ----
        for bg in range(NBG):
            bsl = bass.ts(bg, NB)
            for f in range(FH):
                fsl = bass.ts(f, 128)
                p = pmm.tile([128, NB], F32, tag="mm")
                for d in range(DP2):
                    nc.tensor.matmul(p[:], w3_sb[:, d, :, fsl],
                                     s2[:, 2 * d:2 * d + 2, bsl],
                                     start=(d == 0), stop=(d == DP2 - 1),
                                     perf_mode=mybir.MatmulPerfMode.DoubleRow)
                nc.scalar.activation(s3[:, f, bsl], p[:],
                                     mybir.ActivationFunctionType.Sign,
                                     bias=nu3_sb[:, f:f + 1], scale=1.0)

        # ---- fc4 + log_softmax ----
        for bg in range(NBG):
            bsl = bass.ts(bg, NB)
            p4 = p4p.tile([C, NB], F32, tag="p4")
            for kc in range(FH):
                nc.tensor.matmul(p4[:], w4_sb[:, kc, :], s3[:, kc, bsl],
                                 start=(kc == 0), stop=(kc == FH - 1))
            # logits (feature-major) = raw4 + b4 : single fp32 rounding,
            # bit-identical to the reference's h @ W.T + b4
            lg = small.tile([C, NB], F32, tag="lg")
            nc.vector.tensor_scalar(lg[:], p4[:], b4_sb[:], None,
                                    mybir.AluOpType.add)
            for t in range(NB // 128):
                pt = ptp.tile([128, C], F32, tag="pt")
                nc.tensor.transpose(pt[:], lg[:, bass.ts(t, 128)],
                                    ident[:C, :C])
                l = small.tile([128, C], F32, tag="l")
                nc.vector.tensor_copy(l[:], pt[:])
                mx = small.tile([128, 1], F32, tag="mx")
                nc.vector.tensor_reduce(out=mx[:], in_=l[:],
                                        op=mybir.AluOpType.max,
                                        axis=mybir.AxisListType.X, negate=True)
                ex = small.tile([128, C], F32, tag="ex")
                nc.scalar.activation(ex[:], l[:],
                                     mybir.ActivationFunctionType.Exp,
                                     bias=mx[:], scale=1.0)
                sm = small.tile([128, 1], F32, tag="sm")
                nc.vector.tensor_reduce(out=sm[:], in_=ex[:],
                                        op=mybir.AluOpType.add,
                                        axis=mybir.AxisListType.X)
                lsm = small.tile([128, 1], F32, tag="lsm")
                nc.scalar.activation(lsm[:], sm[:],
                                     mybir.ActivationFunctionType.Ln)
                off = small.tile([128, 1], F32, tag="off")
                nc.vector.tensor_tensor(out=off[:], in0=mx[:], in1=lsm[:],
                                        op=mybir.AluOpType.subtract)
                ot = small.tile([128, C], F32, tag="ot")
                nc.vector.tensor_scalar(ot[:], l[:], off[:], None,
                                        mybir.AluOpType.add)
                nc.sync.dma_start(out[bg * NB + t * 128:bg * NB + (t + 1) * 128, :],
                                  ot[:])

    if do_compile:
        nc.compile()
    return nc


def _prep_inputs(inputs):
    """Host-side packing: sharding, fp16 split, sign-binarization, fp64
    threshold folding."""
    f64 = np.float64
    x = np.asarray(inputs["x"], np.float32)
    W1s = np.sign(np.asarray(inputs["W1"], np.float32))
    W2s = np.sign(np.asarray(inputs["W2"], np.float32))
    W3s = np.sign(np.asarray(inputs["W3"], np.float32))
    W4s = np.sign(np.asarray(inputs["W4"], np.float32))

    def thr(i):
        g = np.asarray(inputs[f"g{i}"], f64)
        be = np.asarray(inputs[f"be{i}"], f64)
        m = np.asarray(inputs[f"m{i}"], f64)
        v = np.asarray(inputs[f"v{i}"], f64)
        return m - be * np.sqrt(v + BN_EPS) / g

    b1 = np.asarray(inputs["b1"], f64)
    b2 = np.asarray(inputs["b2"], f64)
    b3 = np.asarray(inputs["b3"], f64)
    b4 = np.asarray(inputs["b4"], np.float32)

    # layer 1: input real x, output +-1 via Sign(raw1 - u1); bias = -u1
    u1 = (thr(1) - b1).astype(np.float32)
    # layer 2: input +-1 => h2 = raw2 + b2; output {0,1} via raw2 >= u2
    u2v = (thr(2) - b2).astype(np.float32)
    # layer 3: input {0,1} => h3 = 2*raw3 - rowsum3 + b3; output +-1 via
    # Sign(raw3 - u3)
    rowsum3 = W3s.astype(f64).sum(axis=1)
    u3 = ((rowsum3 - b3 + thr(3)) / 2.0).astype(np.float32)
    # layer 4: input +-1 => logits = raw4 + b4

    # fc1 operands: fp16 hi/lo split of x, K = [hi(784) | lo(784) | pad]
    x_hi = x.astype(np.float16)
    x_lo = (x - x_hi.astype(np.float32)).astype(np.float16)
    w1cat = np.zeros((K1P, H), np.float16)
    w1cat[:D_IN] = W1s.T.astype(np.float16)
    w1cat[D_IN:2 * D_IN] = W1s.T.astype(np.float16)
    w1_arr = w1cat.reshape(KC1, 128, H)

    xcatT = np.zeros((K1P, B), np.float16)
    xcatT[:D_IN] = x_hi.T
    xcatT[D_IN:2 * D_IN] = x_lo.T

    w2_arr = np.ascontiguousarray(
        W2s.T.reshape(DP2, 2, 128, H).transpose(0, 2, 1, 3)).astype(
            ml_dtypes.float8_e4m3)
    w3_arr = np.ascontiguousarray(
        W3s.T.reshape(DP2, 2, 128, H).transpose(0, 2, 1, 3)).astype(
            ml_dtypes.float8_e4m3)
    w4_arr = np.ascontiguousarray(W4s.T.reshape(FH, 128, C)).astype(
        ml_dtypes.float8_e4m3)

    nu1_arr = np.ascontiguousarray(-u1.reshape(FH, 128).T)
    u2_arr = np.ascontiguousarray(u2v.reshape(FH, 128).T)
    nu3_arr = np.ascontiguousarray(-u3.reshape(FH, 128).T)
    b4_arr = np.ascontiguousarray(b4.reshape(C, 1))

    shared = {"w1": w1_arr, "w2": w2_arr, "w3": w3_arr, "w4": w4_arr,
              "nu1": nu1_arr, "u2": u2_arr, "nu3": nu3_arr, "b4": b4_arr}
    in_maps = []
    for c in range(N_CORES):
        xc = np.ascontiguousarray(
            xcatT[:, c * BS:(c + 1) * BS]).reshape(KC1, 128, BS)
        in_maps.append({"xcat": xc, **shared})
    return in_maps


def kernel(**inputs):
    if "nc" not in _CACHE:
        _CACHE["nc"] = _build_program()
    nc = _CACHE["nc"]
    in_maps = _prep_inputs(inputs)
    res = run_bass_kernel_spmd(nc, in_maps, list(range(N_CORES)))
    return np.concatenate([res.results[c]["out"] for c in range(N_CORES)],
                          axis=0).astype(np.float32)
